# revision 2
# baseline (speedup 1.0000x reference)
"""BandSplit kernel for Trainium2 (8 NeuronCores, SPMD data-parallel).

Math: the (deterministic) melbank partitions the 1025 STFT bins into 257
contiguous segments (widths 1/4/8/8/1), all mel weights are 1.0, so

    out[b,c,t,k,o] = sum_{f in seg(k)} sum_i x[b,c,t,f,i]*pre_w[i,f,o] + pre_b[k,o]

Sharding: data-parallel over the 8 (b,c) pairs, one per core.
Per core: 256 tokens; out (256, 257, 128) fp32 (33.7 MB -> memory bound).

Device strategy: per-band segment matmuls on the PE, packed 2-4 bands per
matmul as a block-diagonal rhs (K = sum 2w + 1 bias ones-row, N = nb*128),
with lhsT = transposed token-major x slices ("x_mm" layout built on host,
32-aligned partition offsets so several matmuls pack into one 128-row
column group). dtype float32r: 1 cycle/column at N>=256 with ~2e-4 rel err.
PSUM -> SBUF copies alternate VectorE/ScalarE; SBUF -> DRAM via HWDGE DMA.
"""

import numpy as np

import concourse.bacc as bacc
import concourse.mybir as mybir
from concourse.tile import TileContext
from concourse.bass_utils import run_bass_kernel_spmd

# ---------------------------------------------------------------- structure

B, C, T, NF, IN_CH = 4, 2, 256, 1025, 2
N_BANDS, OUT_CH = 257, 128
N_CORES = 8
TOK = 256           # tokens per core (= T; one (b,c) pair per core)
HALVES = 2          # 128-token tiles


def _segments():
    segs = []
    for k in range(N_BANDS):
        if k < 128:
            segs.append((k, 1))
        elif k < 160:
            segs.append((128 + 4 * (k - 128), 4))
        elif k < 192:
            segs.append((256 + 8 * (k - 160), 8))
        elif k < 256:
            segs.append((512 + 8 * (k - 192), 8))
        else:
            segs.append((1024, 1))
    return segs


SEGS = _segments()


def _build_plan():
    """List of matmul descriptors + x/w geometry.

    Each mm: bands (2-4 contiguous), x-group g (256-token column group in
    the x SBUF tensor), partition offset off (32-aligned), K rows
    (sum 2w + 1 ones-row for the bias), N output cols, wcol (col start of
    its W region), outcol (col start in the (256, 257*128) output).
    """
    plan = []
    # class A: width-1 bands 0..127 as 4-band mms (K=9), plus band 256 (K=3)
    for a in range(33):
        bands = [256] if a == 32 else list(range(4 * a, 4 * a + 4))
        plan.append(dict(
            bands=bands, g=a // 4, off=32 * (a % 4),
            K=sum(2 * SEGS[k][1] for k in bands) + 1,
            N=128 * len(bands),
            wcol=512 * (a // 4) if a < 32 else 4096,
            outcol=128 * bands[0],
        ))
    # class B: width-4 bands 128..159 as 4-band mms (K=33)
    for b in range(8):
        bands = list(range(128 + 4 * b, 128 + 4 * b + 4))
        plan.append(dict(
            bands=bands, g=9 + b // 2, off=64 * (b % 2),
            K=33, N=512,
            wcol=4224 + 512 * (b // 2),
            outcol=128 * bands[0],
        ))
    # class C: width-8 bands 160..255 as 2-band mms (K=33)
    for c in range(48):
        bands = [160 + 2 * c, 160 + 2 * c + 1]
        plan.append(dict(
            bands=bands, g=13 + c // 2, off=64 * (c % 2),
            K=33, N=256,
            wcol=4224 + 2048 + 256 * (c // 2),
            outcol=128 * bands[0],
        ))
    return plan


PLAN = _build_plan()
NG = 37                      # x column groups
XCOLS = NG * TOK             # 9472
WCOLS = 4224 + 2048 + 6144   # 12416
OCOLS = N_BANDS * OUT_CH     # 32896


def _xmm_index():
    """Fancy-index arrays to build x_mm from xt (2050, TOK): src row f*2+i,
    dst (group, partition row). Plus the ones-row positions."""
    src, dstg, dstr, og, orow = [], [], [], [], []
    for mm in PLAN:
        r = 0
        for k in mm["bands"]:
            f0, w = SEGS[k]
            for l in range(w):
                for i in range(IN_CH):
                    src.append((f0 + l) * 2 + i)
                    dstg.append(mm["g"])
                    dstr.append(mm["off"] + r)
                    r += 1
        og.append(mm["g"])
        orow.append(mm["off"] + r)
    return (np.array(src), np.array(dstg), np.array(dstr),
            np.array(og), np.array(orow))


_XSRC, _XDG, _XDR, _XOG, _XOR = _xmm_index()

# ---------------------------------------------------------------- host prep


def _build_wmm(pre_w, pre_b):
    """(128, WCOLS) fp32: per-mm block-diagonal weights + bias ones-row."""
    wmm = np.zeros((128, WCOLS), dtype=np.float32)
    for mm in PLAN:
        off, wc = mm["off"], mm["wcol"]
        r = 0
        for j, k in enumerate(mm["bands"]):
            f0, w = SEGS[k]
            cols = slice(wc + 128 * j, wc + 128 * (j + 1))
            for l in range(w):
                for i in range(IN_CH):
                    wmm[off + r, cols] = pre_w[i, f0 + l, :]
                    r += 1
            wmm[off + mm["K"] - 1, cols] = pre_b[k, :]
    return wmm


def _build_xmm(x_core):
    """x_core (TOK, NF, IN_CH) -> (128, XCOLS) fp32 packed lhsT layout."""
    xt = np.ascontiguousarray(x_core.reshape(TOK, NF * IN_CH).T)  # (2050, TOK)
    xmm = np.zeros((NG, 128, TOK), dtype=np.float32)
    xmm[_XDG, _XDR, :] = xt[_XSRC, :]
    xmm[_XOG, _XOR, :] = 1.0
    return np.ascontiguousarray(xmm.transpose(1, 0, 2)).reshape(128, XCOLS)


# ---------------------------------------------------------------- device

_PROGRAM = None


def _build_program():
    global _PROGRAM
    if _PROGRAM is not None:
        return _PROGRAM

    nc = bacc.Bacc("TRN2", target_bir_lowering=False)
    f32 = mybir.dt.float32
    f32r = mybir.dt.float32r
    xin = nc.dram_tensor("xmm", [128, XCOLS], f32r, kind="ExternalInput")
    win = nc.dram_tensor("wmm", [128, WCOLS], f32r, kind="ExternalInput")
    out = nc.dram_tensor("out", [TOK, OCOLS], f32, kind="ExternalOutput")

    X_CHUNK = 5 * TOK      # 5 groups per load DMA
    W_CHUNK = 1552

    with TileContext(nc) as tc:
        with (
            tc.tile_pool(name="xw", bufs=1) as xw_pool,
            tc.tile_pool(name="stage", bufs=8) as stage_pool,
            tc.tile_pool(name="psum", bufs=8, space="PSUM") as psum_pool,
        ):
            x_sb = xw_pool.tile([128, XCOLS], f32r, tag="x")
            w_sb = xw_pool.tile([128, WCOLS], f32r, tag="w")
            for a in range(0, XCOLS, X_CHUNK):
                b_ = min(a + X_CHUNK, XCOLS)
                nc.sync.dma_start(out=x_sb[:, a:b_], in_=xin.ap()[:, a:b_])
            for a in range(0, WCOLS, W_CHUNK):
                b_ = min(a + W_CHUNK, WCOLS)
                nc.sync.dma_start(out=w_sb[:, a:b_], in_=win.ap()[:, a:b_])

            ncopy = 0
            for h in range(HALVES):
                tcol = h * 128
                for mm in PLAN:
                    off, K, N = mm["off"], mm["K"], mm["N"]
                    gcol = mm["g"] * TOK + tcol
                    ps = psum_pool.tile([128, N], f32, tag="ps")
                    nc.tensor.matmul(
                        ps[:],
                        x_sb[off:off + K, gcol:gcol + 128],
                        w_sb[off:off + K, mm["wcol"]:mm["wcol"] + N],
                        start=True, stop=True,
                        tile_position=(off, 0),
                    )
                    sb = stage_pool.tile([128, N], f32, tag="st")
                    if ncopy % 2 == 0:
                        nc.vector.tensor_copy(sb[:], ps[:])
                    else:
                        nc.scalar.copy(sb[:], ps[:])
                    ncopy += 1
                    nc.sync.dma_start(
                        out=out.ap()[tcol:tcol + 128,
                                     mm["outcol"]:mm["outcol"] + N],
                        in_=sb[:],
                    )

    nc.compile()
    _PROGRAM = nc
    return nc


# ---------------------------------------------------------------- entry

LAST_RESULTS = None  # BassKernelResults of the most recent run (for test.py)


def kernel(x, pre_w, pre_b, _trace=False):
    global LAST_RESULTS
    x = np.asarray(x, dtype=np.float32)
    pre_w = np.asarray(pre_w, dtype=np.float32)
    pre_b = np.asarray(pre_b, dtype=np.float32)
    assert x.shape == (B, C, T, NF, IN_CH), x.shape

    nc = _build_program()
    wmm = _build_wmm(pre_w, pre_b)
    in_maps = []
    for core in range(N_CORES):
        b_, c_ = divmod(core, C)
        in_maps.append({"xmm": _build_xmm(x[b_, c_]), "wmm": wmm})

    res = run_bass_kernel_spmd(
        nc, in_maps, core_ids=list(range(N_CORES)), trace=_trace,
    )
    LAST_RESULTS = res

    out = np.empty((B, C, T, N_BANDS, OUT_CH), dtype=np.float32)
    for core in range(N_CORES):
        b_, c_ = divmod(core, C)
        out[b_, c_] = res.results[core]["out"].reshape(T, N_BANDS, OUT_CH)
    return out
